# revision 12
# baseline (speedup 1.0000x reference)
"""DecoderRNN Trainium2 kernel (v3).

Math (reference):
    emb = embed_table[captions]                      # (B, 31, E)
    inputs = concat([features[:,None,:], emb], 1)    # (B, T=32, E)
    xproj = inputs @ Wi + bi                         # (B, T, H)
    h_t = tanh(xproj_t + h_{t-1} @ Wh + bh)          # scan over T
    out = hs @ Wy + by                               # (B, T, V)

Distribution: vocab-parallel output projection across 8 cores (Wy/by sharded
by 1250 columns); the embedding gather, input GEMM and serial RNN are
replicated full-batch on every core. No collectives.

Structure (evidence from HW traces of v1/v2):
  - The PE clock has p-states: ~2.4GHz only under sustained streaming,
    ~1.2GHz when choppy.  The RNN phase is inherently choppy (serial tanh
    chain), so NO projection work is interleaved into it -- the projection
    runs as one solid post-RNN block where the clock ramps and fp16 streams
    2 cols/cycle.
  - RNN step, k-outer order: matmuls contracting h-chunks k=0,1 depend only
    on the previous step's first tanh half, so they execute under the second
    half's tanh; per-step serial chain ~= 2 tanh latencies (~0.8us).
  - xproj_t is injected into PSUM by an identity-stationary matmul
    (start=True) on the PE itself; the Wh matmuls accumulate on top and tanh
    reads PSUM directly.  (A DVE prefill of PSUM races the PE accumulate on
    hardware -- cross-engine PSUM write hazard.)
  - The embedding gather (gpsimd software DGE) is the critical early path:
    wy/by loads are emitted on the scalar queue behind the first GEMM
    activation so their 1.9MB does not steal DMA bandwidth from it.
  - Embedding transpose on the PE (matmul is_transpose): chunk 0 up front
    (PSUM->SBUF copies alternate DVE/scalar), chunks 1-3 spread two
    transposes per RNN step (copies DVE, later chunks gpsimd).
  - Weights host-pre-arranged so every DMA is contiguous per partition.
  - Output fp16, one DMA per 128-row tile.
"""

import sys

sys.path.insert(0, "/opt/trn_rl_repo")

from contextlib import ExitStack

import numpy as np

import concourse.bass as bass
import concourse.mybir as mybir
import concourse.tile as tile
from concourse import bacc
from concourse.bass import ts
from concourse.bass_utils import run_bass_kernel_spmd
from concourse.masks import make_identity

B, T, E, H, V = 64, 32, 512, 512, 10000
NCORES = 8
VS = V // NCORES          # vocab shard per core
BT = B * T                # 2048 rows, t-major: row = t*64 + b
P = 128
KE = E // P               # 4 contraction chunks over E
KH = H // P               # 4 contraction chunks over H
MT = H // P               # 4 output chunks of H
NBT = BT // P             # 16 bt tiles
F32 = mybir.dt.float32
F16 = mybir.dt.float16

# projection N-chunks (psum bank holds 512 fp32 per partition)
VCHUNKS = [(0, 512), (512, 512), (1024, 226)]
assert sum(n for _, n in VCHUNKS) == VS


def build_program() -> bass.Bass:
    nc = bacc.Bacc()

    aug = nc.dram_tensor("aug_table", [V + B, E], F16, kind="ExternalInput")
    idx = nc.dram_tensor("idx", [P, NBT], mybir.dt.int32, kind="ExternalInput")
    wi = nc.dram_tensor("wi", [P, KE * H], F16, kind="ExternalInput")
    wh = nc.dram_tensor("wh", [P, KH * MT * P], F16, kind="ExternalInput")
    bias = nc.dram_tensor("bias", [P, MT], F32, kind="ExternalInput")  # bi+bh
    wy = nc.dram_tensor("wy", [P, KH * VS], F16, kind="ExternalInput")
    byr = nc.dram_tensor("byr", [P, VS], F32, kind="ExternalInput")
    out = nc.dram_tensor("out", [BT, VS], F16, kind="ExternalOutput")

    with ExitStack() as ctx:
        tc = ctx.enter_context(tile.TileContext(nc))
        persist = ctx.enter_context(tc.tile_pool(name="persist", bufs=1))
        osb_pool = ctx.enter_context(tc.tile_pool(name="osb", bufs=2))
        tp_psum = ctx.enter_context(tc.tile_pool(name="tp_ps", bufs=3, space="PSUM"))
        gm_psum = ctx.enter_context(tc.tile_pool(name="gm_ps", bufs=3, space="PSUM"))
        rn_psum = ctx.enter_context(tc.tile_pool(name="rn_ps", bufs=2, space="PSUM"))

        # ---- idx first: it gates the gather, the kernel's critical early path
        idx_sb = persist.tile([P, NBT], mybir.dt.int32, tag="idx")
        nc.sync.dma_start(out=idx_sb[:], in_=idx[:])
        ident = persist.tile([P, P], F16, tag="ident")
        make_identity(nc, ident[:])

        # ---- early weights (contiguous loads; only what the front end needs)
        wi_sb = persist.tile([P, KE, H], F16, tag="wi")
        nc.sync.dma_start(out=wi_sb[:], in_=wi[:].rearrange("p (k h) -> p k h", k=KE))
        wh_sb = persist.tile([P, KH, MT, P], F16, tag="wh")
        nc.sync.dma_start(
            out=wh_sb[:], in_=wh[:].rearrange("p (k m q) -> p k m q", k=KH, m=MT)
        )
        bias_sb = persist.tile([P, MT], F32, tag="bias")
        nc.sync.dma_start(out=bias_sb[:], in_=bias[:])

        # ---- persistent activations
        nat = persist.tile([P, NBT, E], F16, tag="nat")          # gathered rows
        inputsT = persist.tile([P, KE, BT], F16, tag="inputsT")  # E-on-partitions
        xpT = persist.tile([P, T, MT * B], F16, tag="xpT")       # xproj + bias
        hsT = persist.tile([P, MT, (T + 1) * B], F16, tag="hsT") # h states, slot0=0
        wy_sb = persist.tile([P, KH, VS], F16, tag="wy")
        by_rep = persist.tile([P, VS], F32, tag="by_rep")
        nc.vector.memset(hsT[:, :, 0:B], 0.0)

        # ---- all 16 indirect gathers up front on gpsimd (software DGE)
        for i in range(NBT):
            nc.gpsimd.indirect_dma_start(
                out=nat[:, i, :],
                out_offset=None,
                in_=aug[:],
                in_offset=bass.IndirectOffsetOnAxis(ap=idx_sb[:, i : i + 1], axis=0),
            )

        def emit_transpose(i, k, copy_eng):
            # nat block i, e-chunk k -> inputsT[:, k, 128i:128i+128]
            tp = tp_psum.tile([P, P], F16, tag="tp")
            nc.tensor.matmul(
                tp[:], lhsT=nat[:, i, ts(k, P)], rhs=ident[:], is_transpose=True,
            )
            if copy_eng == "v":
                nc.vector.tensor_copy(inputsT[:, k, ts(i, P)], tp[:])
            elif copy_eng == "s":
                nc.scalar.activation(
                    inputsT[:, k, ts(i, P)], tp[:],
                    mybir.ActivationFunctionType.Identity,
                )
            else:
                nc.gpsimd.tensor_copy(inputsT[:, k, ts(i, P)], tp[:])

        def emit_gemm(c):
            # xpT[:, 8c:8c+8, :] = (inputs @ Wi).T + (bi + bh)
            for m in range(MT):
                ps = gm_psum.tile([P, 512], F32, tag="mm")
                for k in range(KE):
                    nc.tensor.matmul(
                        ps[:],
                        lhsT=wi_sb[:, k, ts(m, P)],
                        rhs=inputsT[:, k, ts(c, 512)],
                        start=(k == 0),
                        stop=(k == KE - 1),
                    )
                nc.scalar.activation(
                    xpT[:, 8 * c : 8 * (c + 1), ts(m, B)],
                    ps[:].rearrange("p (t b) -> p t b", b=B),
                    mybir.ActivationFunctionType.Identity,
                    bias=bias_sb[:, m : m + 1],
                )

        def emit_step(t, extra=()):
            # h_t = tanh(xpT[t-1] + Wh.T @ h_{t-1}), k-outer: k=0,1 matmuls
            # depend only on tanh-half-A of step t-1 and run under half-B's
            # tanh.  xpT is injected into PSUM by identity-stationary matmuls
            # (PE-only, no cross-engine PSUM write hazard).
            rps = []
            for half in range(2):
                rp = rn_psum.tile([P, 2 * B], F32, tag="rnn")
                nc.tensor.matmul(
                    rp[:], lhsT=ident[:], rhs=xpT[:, t - 1, ts(half, 2 * B)],
                    start=True, stop=False, skip_group_check=True,
                )
                rps.append(rp)

            def mm(k, m):
                rps[m // 2][:]  # keep tile alive
                nc.tensor.matmul(
                    rps[m // 2][:, ts(m % 2, B)],
                    lhsT=wh_sb[:, k, m, :],
                    rhs=hsT[:, k, (t - 1) * B : t * B],
                    start=False,
                    stop=(k == KH - 1),
                    skip_group_check=True,
                )

            for k in (0, 1):
                for m in range(MT):
                    mm(k, m)
            for m in (0, 1):
                mm(2, m), mm(3, m)
            nc.scalar.activation(
                hsT[:, 0:2, t * B : (t + 1) * B],
                rps[0][:].rearrange("p (m b) -> p m b", b=B),
                mybir.ActivationFunctionType.Tanh,
            )
            for m in (2, 3):
                mm(2, m), mm(3, m)
            for fn in extra:  # transposes for the next chunk fill the tanh wait
                fn()
            nc.scalar.activation(
                hsT[:, 2:4, t * B : (t + 1) * B],
                rps[1][:].rearrange("p (m b) -> p m b", b=B),
                mybir.ActivationFunctionType.Tanh,
            )

        # ---- front end: chunk 0 transposes (copies alternate DVE/scalar),
        # GEMM chunk 0
        for i in range(4):
            for k in range(KE):
                emit_transpose(i, k, "v" if (i * KE + k) % 2 == 0 else "s")
        emit_gemm(0)

        # wy/by loads ride the scalar queue behind the first GEMM activations:
        # they start ~20us in, long before the projection needs them, and do
        # not steal DMA bandwidth from the gather.
        nc.scalar.dma_start(out=wy_sb[:], in_=wy[:].rearrange("p (k v) -> p k v", k=KH))
        nc.scalar.dma_start(out=by_rep[:], in_=byr[:])

        # ---- RNN: 32 steps; chunk c+1's transposes spread 2/step (copies on
        # DVE for chunk 1 while gpsimd still gathers, gpsimd after)
        for c in range(KE):
            if c > 0:
                emit_gemm(c)
            for st in range(8):
                t = 8 * c + st + 1
                extra = []
                if c < KE - 1:
                    i = 4 * (c + 1) + st // 2
                    ks = (0, 1) if st % 2 == 0 else (2, 3)
                    eng = "v"
                    extra = [
                        (lambda i=i, k=k, e=eng: emit_transpose(i, k, e))
                        for k in ks
                    ]
                emit_step(t, extra)

        # ---- projection: one solid PE block (clock ramps), adds alternate
        # DVE/gpsimd, one fp16 DMA per 128-row tile
        osb_tiles = {}
        for gi, (i, vc) in enumerate(
            (i, vc) for i in range(NBT) for vc in range(len(VCHUNKS))
        ):
            v0, vn = VCHUNKS[vc]
            if vc == 0:
                osb = osb_pool.tile([P, VS], F32, tag="osb")
                osb_tiles[i] = osb
            pp = gm_psum.tile([P, 512], F32, tag="mm")
            for k in range(KH):
                nc.tensor.matmul(
                    pp[:, :vn],
                    lhsT=hsT[:, k, (2 * i + 1) * B : (2 * i + 1) * B + P],
                    rhs=wy_sb[:, k, v0 : v0 + vn],
                    start=(k == 0),
                    stop=(k == KH - 1),
                )
            osb = osb_tiles[i]
            eng = nc.vector
            eng.tensor_add(osb[:, v0 : v0 + vn], pp[:, :vn], by_rep[:, v0 : v0 + vn])
            if vc == len(VCHUNKS) - 1:
                nc.gpsimd.dma_start(out=out[ts(i, P), :], in_=osb[:])
                del osb_tiles[i]

    nc.compile()
    return nc


def make_in_maps(features, captions, embed_table, Wi, bi, Wh, bh, Wy, by):
    f32, f16 = np.float32, np.float16
    aug = np.concatenate(
        [np.asarray(embed_table, f32), np.asarray(features, f32)], axis=0
    ).astype(f16)
    idx = np.empty((T, B), np.int32)
    idx[0] = V + np.arange(B, dtype=np.int32)
    idx[1:] = np.asarray(captions, np.int64).T.astype(np.int32)
    idx_t = np.ascontiguousarray(idx.reshape(BT).reshape(NBT, P).T)  # [128, 16]

    # host pre-arrangement: weight DMAs become one contiguous run/partition
    wi16 = np.asarray(Wi, f32).astype(f16)      # [E, H]
    wi_h = np.ascontiguousarray(
        wi16.reshape(KE, P, H).transpose(1, 0, 2).reshape(P, KE * H)
    )
    wh16 = np.asarray(Wh, f32).astype(f16)      # [H, H]
    wh_h = np.ascontiguousarray(
        wh16.reshape(KH, P, MT, P).transpose(1, 0, 2, 3).reshape(P, KH * MT * P)
    )
    bias_c = (np.asarray(bi, f32) + np.asarray(bh, f32)).astype(f32)
    bias_h = np.ascontiguousarray(bias_c.reshape(MT, P).T)  # [128, MT]
    wy16 = np.asarray(Wy, f32).astype(f16)      # [H, V]
    by_f = np.asarray(by, f32)

    in_maps = []
    for c in range(NCORES):
        wy_sh = wy16[:, c * VS : (c + 1) * VS]  # [H, VS]
        wy_h = np.ascontiguousarray(
            wy_sh.reshape(KH, P, VS).transpose(1, 0, 2).reshape(P, KH * VS)
        )
        by_sh = by_f[c * VS : (c + 1) * VS]
        byr = np.ascontiguousarray(np.broadcast_to(by_sh, (P, VS)))
        in_maps.append(
            {
                "aug_table": aug,
                "idx": idx_t,
                "wi": wi_h,
                "wh": wh_h,
                "bias": bias_h,
                "wy": wy_h,
                "byr": byr,
            }
        )
    return in_maps


def assemble(core_outs):
    full = np.concatenate([np.asarray(o) for o in core_outs], axis=1)  # [BT, V]
    return np.ascontiguousarray(
        full.reshape(T, B, V).transpose(1, 0, 2).astype(np.float32)
    )


def kernel(**inputs) -> np.ndarray:
    in_maps = make_in_maps(**inputs)
    nc = build_program()
    res = run_bass_kernel_spmd(nc, in_maps, core_ids=list(range(NCORES)))
    return assemble([r["out"] for r in res.results])


# revision 13
# speedup vs baseline: 1.0223x; 1.0223x over previous
"""DecoderRNN Trainium2 kernel (v3).

Math (reference):
    emb = embed_table[captions]                      # (B, 31, E)
    inputs = concat([features[:,None,:], emb], 1)    # (B, T=32, E)
    xproj = inputs @ Wi + bi                         # (B, T, H)
    h_t = tanh(xproj_t + h_{t-1} @ Wh + bh)          # scan over T
    out = hs @ Wy + by                               # (B, T, V)

Distribution: vocab-parallel output projection across 8 cores (Wy/by sharded
by 1250 columns); the embedding gather, input GEMM and serial RNN are
replicated full-batch on every core. No collectives.

Structure (evidence from HW traces of v1/v2):
  - The PE clock has p-states: ~2.4GHz only under sustained streaming,
    ~1.2GHz when choppy.  The RNN phase is inherently choppy (serial tanh
    chain), so NO projection work is interleaved into it -- the projection
    runs as one solid post-RNN block where the clock ramps and fp16 streams
    2 cols/cycle.
  - RNN step, k-outer order: matmuls contracting h-chunks k=0,1 depend only
    on the previous step's first tanh half, so they execute under the second
    half's tanh; per-step serial chain ~= 2 tanh latencies (~0.8us).
  - xproj_t is injected into PSUM by an identity-stationary matmul
    (start=True) on the PE itself; the Wh matmuls accumulate on top and tanh
    reads PSUM directly.  (A DVE prefill of PSUM races the PE accumulate on
    hardware -- cross-engine PSUM write hazard.)
  - The embedding gather (gpsimd software DGE) is the critical early path:
    wy/by loads are emitted on the scalar queue behind the first GEMM
    activation so their 1.9MB does not steal DMA bandwidth from it.
  - Embedding transpose on the PE (matmul is_transpose): chunk 0 up front
    (PSUM->SBUF copies alternate DVE/scalar), chunks 1-3 spread two
    transposes per RNN step (copies DVE, later chunks gpsimd).
  - Weights host-pre-arranged so every DMA is contiguous per partition.
  - Output fp16, one DMA per 128-row tile.
"""

import sys

sys.path.insert(0, "/opt/trn_rl_repo")

from contextlib import ExitStack

import numpy as np

import concourse.bass as bass
import concourse.mybir as mybir
import concourse.tile as tile
from concourse import bacc
from concourse.bass import ts
from concourse.bass_utils import run_bass_kernel_spmd
from concourse.masks import make_identity

B, T, E, H, V = 64, 32, 512, 512, 10000
NCORES = 8
VS = V // NCORES          # vocab shard per core
BT = B * T                # 2048 rows, t-major: row = t*64 + b
P = 128
KE = E // P               # 4 contraction chunks over E
KH = H // P               # 4 contraction chunks over H
MT = H // P               # 4 output chunks of H
NBT = BT // P             # 16 bt tiles
F32 = mybir.dt.float32
F16 = mybir.dt.float16

# projection N-chunks (psum bank holds 512 fp32 per partition)
VCHUNKS = [(0, 512), (512, 512), (1024, 226)]
assert sum(n for _, n in VCHUNKS) == VS


def build_program() -> bass.Bass:
    nc = bacc.Bacc(num_swdge_queues=4)

    aug = nc.dram_tensor("aug_table", [V + B, E], F16, kind="ExternalInput")
    idx = nc.dram_tensor("idx", [P, NBT], mybir.dt.int32, kind="ExternalInput")
    wi = nc.dram_tensor("wi", [P, KE * H], F16, kind="ExternalInput")
    wh = nc.dram_tensor("wh", [P, KH * MT * P], F16, kind="ExternalInput")
    bias = nc.dram_tensor("bias", [P, MT], F32, kind="ExternalInput")  # bi+bh
    wy = nc.dram_tensor("wy", [P, KH * VS], F16, kind="ExternalInput")
    byr = nc.dram_tensor("byr", [P, VS], F32, kind="ExternalInput")
    out = nc.dram_tensor("out", [BT, VS], F16, kind="ExternalOutput")

    with ExitStack() as ctx:
        tc = ctx.enter_context(tile.TileContext(nc))
        persist = ctx.enter_context(tc.tile_pool(name="persist", bufs=1))
        osb_pool = ctx.enter_context(tc.tile_pool(name="osb", bufs=2))
        tp_psum = ctx.enter_context(tc.tile_pool(name="tp_ps", bufs=2, space="PSUM"))
        gm_psum = ctx.enter_context(tc.tile_pool(name="gm_ps", bufs=3, space="PSUM"))
        rn_psum = ctx.enter_context(tc.tile_pool(name="rn_ps", bufs=3, space="PSUM"))

        # ---- idx first: it gates the gather, the kernel's critical early path
        idx_sb = persist.tile([P, NBT], mybir.dt.int32, tag="idx")
        nc.sync.dma_start(out=idx_sb[:], in_=idx[:])
        ident = persist.tile([P, P], F16, tag="ident")
        make_identity(nc, ident[:])

        # ---- early weights (contiguous loads; only what the front end needs)
        wi_sb = persist.tile([P, KE, H], F16, tag="wi")
        nc.sync.dma_start(out=wi_sb[:], in_=wi[:].rearrange("p (k h) -> p k h", k=KE))
        wh_sb = persist.tile([P, KH, MT, P], F16, tag="wh")
        nc.sync.dma_start(
            out=wh_sb[:], in_=wh[:].rearrange("p (k m q) -> p k m q", k=KH, m=MT)
        )
        bias_sb = persist.tile([P, MT], F32, tag="bias")
        nc.sync.dma_start(out=bias_sb[:], in_=bias[:])

        # ---- persistent activations
        nat = persist.tile([P, NBT, E], F16, tag="nat")          # gathered rows
        inputsT = persist.tile([P, KE, BT], F16, tag="inputsT")  # E-on-partitions
        xpT = persist.tile([P, T, MT * B], F16, tag="xpT")       # xproj + bias
        hsT = persist.tile([P, MT, (T + 1) * B], F16, tag="hsT") # h states, slot0=0
        wy_sb = persist.tile([P, KH, VS], F16, tag="wy")
        by_rep = persist.tile([P, VS], F32, tag="by_rep")
        nc.vector.memset(hsT[:, :, 0:B], 0.0)

        # ---- all 16 indirect gathers up front on gpsimd (software DGE)
        for i in range(NBT):
            nc.gpsimd.indirect_dma_start(
                out=nat[:, i, :],
                out_offset=None,
                in_=aug[:],
                in_offset=bass.IndirectOffsetOnAxis(ap=idx_sb[:, i : i + 1], axis=0),
            )

        def emit_transpose(i, k, copy_eng):
            # nat block i, e-chunk k -> inputsT[:, k, 128i:128i+128]
            tp = tp_psum.tile([P, P], F16, tag="tp")
            nc.tensor.matmul(
                tp[:], lhsT=nat[:, i, ts(k, P)], rhs=ident[:], is_transpose=True,
            )
            if copy_eng == "v":
                nc.vector.tensor_copy(inputsT[:, k, ts(i, P)], tp[:])
            elif copy_eng == "s":
                nc.scalar.activation(
                    inputsT[:, k, ts(i, P)], tp[:],
                    mybir.ActivationFunctionType.Identity,
                )
            else:
                nc.gpsimd.tensor_copy(inputsT[:, k, ts(i, P)], tp[:])

        def emit_gemm(c):
            # xpT[:, 8c:8c+8, :] = (inputs @ Wi).T + (bi + bh)
            for m in range(MT):
                ps = gm_psum.tile([P, 512], F32, tag="mm")
                for k in range(KE):
                    nc.tensor.matmul(
                        ps[:],
                        lhsT=wi_sb[:, k, ts(m, P)],
                        rhs=inputsT[:, k, ts(c, 512)],
                        start=(k == 0),
                        stop=(k == KE - 1),
                    )
                nc.scalar.activation(
                    xpT[:, 8 * c : 8 * (c + 1), ts(m, B)],
                    ps[:].rearrange("p (t b) -> p t b", b=B),
                    mybir.ActivationFunctionType.Identity,
                    bias=bias_sb[:, m : m + 1],
                )

        def emit_step(t, extra=()):
            # h_t = tanh(xpT[t-1] + Wh.T @ h_{t-1}) in one [128,256] PSUM
            # tile: one identity-stationary inject (start=True, runs early --
            # its bank was freed by tanh at t-2), 16 Wh matmuls accumulate,
            # one tanh reads PSUM and writes all four h-chunks.
            rp = rn_psum.tile([P, MT * B], F32, tag="rnn")
            nc.tensor.matmul(
                rp[:], lhsT=ident[:], rhs=xpT[:, t - 1, :],
                start=True, stop=False, skip_group_check=True,
            )
            for k in range(KH):
                for m in range(MT):
                    nc.tensor.matmul(
                        rp[:, ts(m, B)],
                        lhsT=wh_sb[:, k, m, :],
                        rhs=hsT[:, k, (t - 1) * B : t * B],
                        start=False,
                        stop=(k == KH - 1),
                        skip_group_check=True,
                    )
            for fn in extra:  # next chunk's transposes fill the tanh wait
                fn()
            nc.scalar.activation(
                hsT[:, :, t * B : (t + 1) * B],
                rp[:].rearrange("p (m b) -> p m b", b=B),
                mybir.ActivationFunctionType.Tanh,
            )

        # ---- front end: chunk 0 transposes (copies alternate DVE/scalar),
        # GEMM chunk 0
        for i in range(4):
            for k in range(KE):
                emit_transpose(i, k, "v" if (i * KE + k) % 2 == 0 else "s")
        emit_gemm(0)

        # wy/by loads ride the scalar queue behind the first GEMM activations:
        # they start ~20us in, long before the projection needs them, and do
        # not steal DMA bandwidth from the gather.
        nc.scalar.dma_start(out=wy_sb[:], in_=wy[:].rearrange("p (k v) -> p k v", k=KH))
        nc.scalar.dma_start(out=by_rep[:], in_=byr[:])

        # ---- RNN: 32 steps; chunk c+1's transposes spread 2/step (copies on
        # DVE for chunk 1 while gpsimd still gathers, gpsimd after)
        for c in range(KE):
            if c > 0:
                emit_gemm(c)
            for st in range(8):
                t = 8 * c + st + 1
                extra = []
                if c < KE - 1:
                    i = 4 * (c + 1) + st // 2
                    ks = (0, 1) if st % 2 == 0 else (2, 3)
                    eng = "v"
                    extra = [
                        (lambda i=i, k=k, e=eng: emit_transpose(i, k, e))
                        for k in ks
                    ]
                emit_step(t, extra)

        # ---- projection: one solid PE block (clock ramps), adds alternate
        # DVE/gpsimd, one fp16 DMA per 128-row tile
        osb_tiles = {}
        for gi, (i, vc) in enumerate(
            (i, vc) for i in range(NBT) for vc in range(len(VCHUNKS))
        ):
            v0, vn = VCHUNKS[vc]
            if vc == 0:
                osb = osb_pool.tile([P, VS], F16, tag="osb")
                osb_tiles[i] = osb
            pp = gm_psum.tile([P, 512], F32, tag="mm")
            for k in range(KH):
                nc.tensor.matmul(
                    pp[:, :vn],
                    lhsT=hsT[:, k, (2 * i + 1) * B : (2 * i + 1) * B + P],
                    rhs=wy_sb[:, k, v0 : v0 + vn],
                    start=(k == 0),
                    stop=(k == KH - 1),
                )
            osb = osb_tiles[i]
            eng = nc.vector
            eng.tensor_add(osb[:, v0 : v0 + vn], pp[:, :vn], by_rep[:, v0 : v0 + vn])
            if vc == len(VCHUNKS) - 1:
                nc.sync.dma_start(out=out[ts(i, P), :], in_=osb[:])
                del osb_tiles[i]

    nc.compile()
    return nc


def make_in_maps(features, captions, embed_table, Wi, bi, Wh, bh, Wy, by):
    f32, f16 = np.float32, np.float16
    aug = np.concatenate(
        [np.asarray(embed_table, f32), np.asarray(features, f32)], axis=0
    ).astype(f16)
    idx = np.empty((T, B), np.int32)
    idx[0] = V + np.arange(B, dtype=np.int32)
    idx[1:] = np.asarray(captions, np.int64).T.astype(np.int32)
    idx_t = np.ascontiguousarray(idx.reshape(BT).reshape(NBT, P).T)  # [128, 16]

    # host pre-arrangement: weight DMAs become one contiguous run/partition
    wi16 = np.asarray(Wi, f32).astype(f16)      # [E, H]
    wi_h = np.ascontiguousarray(
        wi16.reshape(KE, P, H).transpose(1, 0, 2).reshape(P, KE * H)
    )
    wh16 = np.asarray(Wh, f32).astype(f16)      # [H, H]
    wh_h = np.ascontiguousarray(
        wh16.reshape(KH, P, MT, P).transpose(1, 0, 2, 3).reshape(P, KH * MT * P)
    )
    bias_c = (np.asarray(bi, f32) + np.asarray(bh, f32)).astype(f32)
    bias_h = np.ascontiguousarray(bias_c.reshape(MT, P).T)  # [128, MT]
    wy16 = np.asarray(Wy, f32).astype(f16)      # [H, V]
    by_f = np.asarray(by, f32)

    in_maps = []
    for c in range(NCORES):
        wy_sh = wy16[:, c * VS : (c + 1) * VS]  # [H, VS]
        wy_h = np.ascontiguousarray(
            wy_sh.reshape(KH, P, VS).transpose(1, 0, 2).reshape(P, KH * VS)
        )
        by_sh = by_f[c * VS : (c + 1) * VS]
        byr = np.ascontiguousarray(np.broadcast_to(by_sh, (P, VS)))
        in_maps.append(
            {
                "aug_table": aug,
                "idx": idx_t,
                "wi": wi_h,
                "wh": wh_h,
                "bias": bias_h,
                "wy": wy_h,
                "byr": byr,
            }
        )
    return in_maps


def assemble(core_outs):
    full = np.concatenate([np.asarray(o) for o in core_outs], axis=1)  # [BT, V]
    return np.ascontiguousarray(
        full.reshape(T, B, V).transpose(1, 0, 2).astype(np.float32)
    )


def kernel(**inputs) -> np.ndarray:
    in_maps = make_in_maps(**inputs)
    nc = build_program()
    res = run_bass_kernel_spmd(nc, in_maps, core_ids=list(range(NCORES)))
    return assemble([r["out"] for r in res.results])
